# revision 17
# baseline (speedup 1.0000x reference)
"""Trainium2 Bass kernel for nn_ListenerModelBertAttCtxHist — ragged v2.

Data-parallel over the batch dim (64 -> 8 cores x 8 slots) PLUS ragged
sequence packing: masked positions contribute exactly zero to the model
output (their attention weight is exp(-1e30)=0 and scores at kept positions
do not depend on masked ones), so the host gathers only the unmasked
positions of each sequence. Batches are sorted by kept-length and assigned
to (slot, core) so each slot's compiled width is the max over its 8 cores;
slot widths are baked into the compiled program (cached per width tuple).
This halves all S-proportional matmul/activation work (~2076 kept vs 4096).

Everything else as v1: bf16 host-packed contiguous DMA, early ReduceScatter
for the sharded W_ctx projection, PE-saturated phase A/B pipeline, DVE
Newton rsqrt for the L2 norm, fused multiply-reduce attention combine.
"""

import numpy as np
import ml_dtypes

import concourse.bacc as bacc
import concourse.mybir as mybir
import concourse.tile as tile
from concourse.bass_utils import run_bass_kernel_spmd
from concourse.masks import make_identity

F32 = mybir.dt.float32
BF16 = mybir.dt.bfloat16
I32 = mybir.dt.int32

B, S, EMB, HID, IMG, ATT, K6, HL = 64, 512, 768, 512, 2048, 512, 6, 20
NCORES = 8
BL = B // NCORES
BK = BL * K6
SHARD = IMG * K6 // NCORES

BF = ml_dtypes.bfloat16
_NC_CACHE = {}

import os
USE_PB = os.environ.get("K_USE_PB", "0") == "1"


def _build_nc(widths):
    W0 = widths[0]
    cum = [0]
    for w in widths:
        cum.append(cum[-1] + w)
    SW = cum[-1]

    nc = bacc.Bacc("TRN2", target_bir_lowering=False, debug=False,
                   num_devices=NCORES)
    AF = mybir.ActivationFunctionType
    OP = mybir.AluOpType

    d_xt = nc.dram_tensor("xt", [128, K6 * SW], BF16, kind="ExternalInput")
    d_we2h = nc.dram_tensor("we2h", [128, K6 * HID], BF16, kind="ExternalInput")
    d_wmm = nc.dram_tensor("wmm", [128, 8 * HID], BF16, kind="ExternalInput")
    d_wa1 = nc.dram_tensor("wa1", [128, 4 * ATT], BF16, kind="ExternalInput")
    d_wa2 = nc.dram_tensor("wa2", [128, 4], BF16, kind="ExternalInput")
    d_whist = nc.dram_tensor("whist", [128, K6 * HID], BF16, kind="ExternalInput")
    d_wsep = nc.dram_tensor("wsep", [128, 16 * HID], BF16, kind="ExternalInput")
    d_wctx = nc.dram_tensor("wctx", [128, 96 * HID], BF16, kind="ExternalInput")
    d_vct = nc.dram_tensor("vct", [128, 96 * BL], BF16, kind="ExternalInput")
    d_sit = nc.dram_tensor("sit", [128, 16 * BK], BF16, kind="ExternalInput")
    d_ph = nc.dram_tensor("ph", [2 * BK, HL * (EMB // 2)], BF16,
                          kind="ExternalInput")
    d_w96 = nc.dram_tensor("w96", [2 * BK, HL], F32, kind="ExternalInput")
    d_bias = nc.dram_tensor("bias", [128, 12], F32, kind="ExternalInput")
    d_rows = nc.dram_tensor("rows", [1, 3 * HID], BF16, kind="ExternalInput")
    d_mterm = nc.dram_tensor("mterm", [1, SW], BF16, kind="ExternalInput")
    d_g48 = nc.dram_tensor("g48", [1, BK], BF16, kind="ExternalInput")
    d_a48 = nc.dram_tensor("a48", [BL, BK], BF16, kind="ExternalInput")
    d_out = nc.dram_tensor("out", [BK, 1], F32, kind="ExternalOutput")

    with tile.TileContext(nc) as tc:
        with (
            tc.tile_pool(name="const", bufs=1) as cw,
            tc.tile_pool(name="repp", bufs=3) as repp,
            tc.tile_pool(name="mmp", bufs=8) as mmp,
            tc.tile_pool(name="aTp", bufs=2) as aTp,
            tc.tile_pool(name="wbcp", bufs=2) as wbcp,
            tc.tile_pool(name="scrp", bufs=2) as scrp,
            tc.tile_pool(name="bp", bufs=2) as bp,
            tc.tile_pool(name="pbig", bufs=4, space="PSUM") as pbig,
            tc.tile_pool(name="pacc", bufs=2, space="PSUM") as pacc,
            tc.tile_pool(name="psmall", bufs=2, space="PSUM") as psmall,
        ):
            # ======== sync queue: small consts, then streamed W_ctx ========
            vct = cw.tile([128, 96, BL], BF16)
            nc.sync.dma_start(vct[:], d_vct.ap().rearrange(
                "p (a b) -> p a b", a=96))
            biasp = cw.tile([128, 12], F32)
            nc.sync.dma_start(biasp[:], d_bias.ap())
            rows = cw.tile([1, 3 * HID], BF16)
            nc.sync.dma_start(rows[:], d_rows.ap())
            mterm = cw.tile([1, SW], BF16)
            nc.sync.dma_start(mterm[:], d_mterm.ap())
            w96 = cw.tile([2 * BK, HL], F32)
            nc.sync.dma_start(w96[:], d_w96.ap())
            g48 = cw.tile([1, BK], BF16)
            nc.sync.dma_start(g48[:], d_g48.ap())
            a48 = cw.tile([BL, BK], BF16)
            nc.sync.dma_start(a48[:], d_a48.ap())

            # ======== scalar HWDGE queue: token chunks + weights ===========
            we2h = cw.tile([128, K6, HID], BF16)
            nc.scalar.dma_start(we2h[:], d_we2h.ap().rearrange(
                "p (a h) -> p a h", a=K6))
            xt = cw.tile([128, K6 * SW], BF16)

            def xt_view(b):
                return xt[:, K6 * cum[b]:K6 * cum[b + 1]].rearrange(
                    "p (a n) -> p a n", a=K6)

            nc.scalar.dma_start(
                xt[:, :K6 * cum[1]], d_xt.ap()[:, :K6 * cum[1]])
            nc.scalar.dma_start(
                xt[:, K6 * cum[1]:K6 * cum[2]],
                d_xt.ap()[:, K6 * cum[1]:K6 * cum[2]])
            wmm = cw.tile([128, 8, HID], BF16)
            nc.scalar.dma_start(wmm[:], d_wmm.ap().rearrange(
                "p (a h) -> p a h", a=8))
            for b in range(2, BL):
                nc.scalar.dma_start(
                    xt[:, K6 * cum[b]:K6 * cum[b + 1]],
                    d_xt.ap()[:, K6 * cum[b]:K6 * cum[b + 1]])
            ones8th = cw.tile([1, B], BF16)
            nc.gpsimd.memset(ones8th[:], 1.0 / NCORES)
            ones48 = cw.tile([1, BK], BF16)
            nc.gpsimd.memset(ones48[:], 1.0)
            ones1 = cw.tile([1, 128], BF16)
            nc.gpsimd.memset(ones1[:], 1.0)
            identf = cw.tile([128, 128], F32)
            make_identity(nc, identf[:])
            identb = cw.tile([128, 128], BF16)
            make_identity(nc, identb[:])

            # sep/hist bulk loads on gpsimd
            phb = cw.tile([2 * BK, HL, EMB // 2], BF16)
            nc.gpsimd.dma_start(phb[:], d_ph.ap().rearrange(
                "p (l e) -> p l e", l=HL))
            whist = cw.tile([128, K6, HID], BF16)
            nc.gpsimd.dma_start(whist[:], d_whist.ap().rearrange(
                "p (a h) -> p a h", a=K6))
            wa1 = cw.tile([128, 4, ATT], BF16)
            nc.gpsimd.dma_start(wa1[:], d_wa1.ap().rearrange(
                "p (a h) -> p a h", a=4))
            wa2 = cw.tile([128, 4], BF16)
            nc.gpsimd.dma_start(wa2[:], d_wa2.ap())
            sit = cw.tile([128, 16, BK], BF16)
            nc.gpsimd.dma_start(sit[:], d_sit.ap().rearrange(
                "p (a n) -> p a n", a=16))
            wsep = cw.tile([128, 16, HID], BF16)
            nc.gpsimd.dma_start(wsep[:], d_wsep.ap().rearrange(
                "p (a h) -> p a h", a=16))

            # ======== helper emitters ======================================
            havg = cw.tile([2 * BK, EMB // 2], F32)
            havgT = cw.tile([128, K6, BK], BF16)
            hproj = cw.tile([BK, HID], F32)
            sep = cw.tile([BK, HID], F32)
            ssq = cw.tile([BK, 1], F32)
            rnorm = cw.tile([BK, 1], F32)

            def emit_hist_dve():
                w_bc = w96[:].unsqueeze(2).broadcast_to(
                    [2 * BK, HL, EMB // 2])
                nc.vector.tensor_tensor(phb[:], phb[:], w_bc, op=OP.mult)
                nc.vector.tensor_reduce(
                    havg[:], phb[:].rearrange("p l e -> p e l"),
                    axis=mybir.AxisListType.X, op=OP.add)

            def emit_havgT():
                for j in range(3):
                    pt96 = psmall.tile([128, 2 * BK], F32, tag="small")
                    nc.tensor.transpose(pt96[:], havg[:, j * 128:(j + 1) * 128],
                                        identf[:2 * BK, :2 * BK])
                    for c in range(2):
                        nc.vector.tensor_copy(havgT[:, c * 3 + j, :],
                                              pt96[:, c * BK:(c + 1) * BK])

            def emit_sep_hist_pe():
                php = pacc.tile([BK, HID], F32, tag="acc")
                nc.tensor.matmul(php[:], g48[:], rows[:, 2 * HID:3 * HID],
                                 start=True, stop=False)
                for et in range(K6):
                    nc.tensor.matmul(php[:], havgT[:, et, :], whist[:, et, :],
                                     start=False, stop=(et == K6 - 1))
                nc.scalar.activation(hproj[:], php[:], AF.Relu)
                psep = pacc.tile([BK, HID], F32, tag="acc")
                nc.tensor.matmul(psep[:], ones48[:], rows[:, HID:2 * HID],
                                 start=True, stop=False)
                for kt in range(16):
                    nc.tensor.matmul(psep[:], sit[:, kt, :], wsep[:, kt, :],
                                     start=False, stop=(kt == 15))
                nc.vector.tensor_tensor(sep[:], psep[:], hproj[:], op=OP.add)
                nc.vector.tensor_scalar_max(sep[:], sep[:], 0.0)

            def emit_norm_dve():
                scr48 = cw.tile([BK, HID], F32)
                nc.scalar.activation(scr48[:], sep[:], AF.Square,
                                     accum_out=ssq[:])
                snorm = cw.tile([BK, 1], F32)
                nc.scalar.activation(snorm[:], ssq[:], AF.Sqrt)
                snormc = cw.tile([BK, 1], F32)
                nc.vector.tensor_scalar_max(snormc[:], snorm[:], 1e-12)
                nc.vector.reciprocal(rnorm[:], snormc[:])

            # ======== phase A ==============================================
            mmAs = {}
            for b in range(BL):
                W = widths[b]
                xv = xt_view(b)
                repsT = repp.tile([128, 4, W0], BF16, tag="repsT",
                                  name=f"repsT{b}")
                for mt in range(4):
                    msl = slice(mt * 128, (mt + 1) * 128)
                    pe = pbig.tile([128, W0], F32, tag="big")
                    for kt in range(K6):
                        nc.tensor.matmul(pe[:, :W], we2h[:, kt, msl],
                                         xv[:, kt, :],
                                         start=(kt == 0), stop=(kt == K6 - 1))
                    nc.scalar.activation(repsT[:, mt, :W], pe[:, :W],
                                         AF.Relu, bias=biasp[:, mt:mt + 1])
                mmA = mmp.tile([128, 4, W0], BF16, tag="mmA", name=f"mmA{b}")
                mmAs[b] = mmA
                for mt in range(4):
                    msl = slice(mt * 128, (mt + 1) * 128)
                    pm = pbig.tile([128, W0], F32, tag="big")
                    for kt in range(4):
                        nc.tensor.matmul(pm[:, :W], wmm[:, kt, msl],
                                         repsT[:, kt, :W],
                                         start=(kt == 0), stop=(kt == 3))
                    if mt % 2 == 0:
                        nc.vector.tensor_copy(mmA[:, mt, :W], pm[:, :W])
                    else:
                        nc.scalar.copy(mmA[:, mt, :W], pm[:, :W])
                if b == 2:
                    emit_hist_dve()
                if b == 4:
                    emit_havgT()
                if b == 5:
                    emit_sep_hist_pe()
                if b == 6:
                    emit_norm_dve()

            # ======== ctx: full-W_ctx projection for own batches ===========
            with tc.tile_pool(name="wctxp", bufs=3) as wctxp:
                pctx = pacc.tile([BL, HID], F32, tag="acc")
                nc.tensor.matmul(pctx[:], ones1[:, :BL], rows[:, 0:HID],
                                 start=True, stop=False)
                for ch in range(8):
                    wct = wctxp.tile([128, 12, HID], BF16, tag="wc")
                    nc.sync.dma_start(
                        wct[:], d_wctx.ap().rearrange(
                            "p (a h) -> p a h", a=96)[:, 12 * ch:12 * ch + 12])
                    for g in range(12):
                        gg = 12 * ch + g
                        nc.tensor.matmul(pctx[:], vct[:, gg, :], wct[:, g, :],
                                         start=False, stop=(gg == 95))
                ctxh = cw.tile([BL, HID], BF16)
                nc.scalar.activation(ctxh[:], pctx[:], AF.Relu)
            ctxT = cw.tile([128, 4, BL], BF16)
            for j in range(4):
                pt = psmall.tile([128, BL], BF16, tag="small")
                nc.tensor.transpose(pt[:], ctxh[:, j * 128:(j + 1) * 128],
                                    identb[:BL, :BL])
                nc.vector.tensor_copy(ctxT[:, j, :], pt[:])
            cbiasT = cw.tile([128, 4, BL], F32)
            for mt in range(4):
                msl = slice(mt * 128, (mt + 1) * 128)
                pcb = psmall.tile([128, BL], F32, tag="small")
                for kt in range(4):
                    nc.tensor.matmul(pcb[:], wmm[:, 4 + kt, msl],
                                     ctxT[:, kt, :],
                                     start=(kt == 0), stop=(kt == 3))
                nc.vector.tensor_scalar(cbiasT[:, mt, :], pcb[:],
                                        biasp[:, 4 + mt:5 + mt], None,
                                        op0=OP.add)

            # ======== phase B ==============================================
            attT = cw.tile([128, 4, BL], F32)
            wbcs = {}

            def emit_attend(b):
                W = widths[b]
                for mt in range(4):
                    scrb = scrp.tile([128, W0], BF16, tag="scrb")
                    nc.vector.tensor_tensor(scrb[:, :W], mmAs[b][:, mt, :W],
                                            wbcs[b][:, :W], op=OP.mult)
                    nc.vector.tensor_reduce(attT[:, mt, b:b + 1],
                                            scrb[:, :W],
                                            axis=mybir.AxisListType.X,
                                            op=OP.add)

            for b in range(BL):
                W = widths[b]
                mmT = mmAs[b]
                for mt in range(1):
                    nc.scalar.activation(mmT[:, mt, :W], mmT[:, mt, :W],
                                         AF.Relu, bias=cbiasT[:, mt, b:b + 1])
                for mt in range(1, 4):
                    nc.vector.tensor_scalar_add(mmT[:, mt, :W], mmT[:, mt, :W],
                                                cbiasT[:, mt, b:b + 1])
                    nc.vector.tensor_scalar_max(mmT[:, mt, :W], mmT[:, mt, :W],
                                                0.0)
                aT = aTp.tile([128, 4, W0], BF16, tag="aT")
                for mt in range(4):
                    msl = slice(mt * 128, (mt + 1) * 128)
                    pa = pbig.tile([128, W0], F32, tag="big")
                    for kt in range(4):
                        nc.tensor.matmul(pa[:, :W], wa1[:, kt, msl],
                                         mmT[:, kt, :W],
                                         start=(kt == 0), stop=(kt == 3))
                    nc.scalar.activation(aT[:, mt, :W], pa[:, :W], AF.Tanh,
                                         bias=biasp[:, 8 + mt:9 + mt])
                psc = psmall.tile([1, W0], F32, tag="small")
                for kt in range(4):
                    nc.tensor.matmul(psc[:, :W], wa2[:, kt:kt + 1],
                                     aT[:, kt, :W],
                                     start=(kt == 0), stop=(kt == 3))
                sc = bp.tile([1, W0], F32, tag="sc")
                nc.vector.tensor_tensor(sc[:, :W], psc[:, :W],
                                        mterm[:, cum[b]:cum[b] + W],
                                        op=OP.add)
                esc = bp.tile([1, W0], F32, tag="esc")
                zsum = bp.tile([1, 1], F32, tag="zsum")
                nc.scalar.activation(esc[:, :W], sc[:, :W], AF.Exp,
                                     accum_out=zsum[:])
                rz = bp.tile([1, 1], F32, tag="rz")
                nc.vector.reciprocal(rz[:], zsum[:])
                attw = bp.tile([1, W0], BF16, tag="attw")
                nc.vector.tensor_scalar_mul(attw[:, :W], esc[:, :W], rz[:])
                wbc = wbcp.tile([128, W0], BF16, tag="wbc")
                wbcs[b] = wbc
                if USE_PB:
                    nc.gpsimd.partition_broadcast(wbc[:, :W], attw[:, :W])
                else:
                    pwb = pbig.tile([128, W0], F32, tag="big")
                    nc.tensor.matmul(pwb[:, :W], ones1[:], attw[:, :W],
                                     start=True, stop=True)
                    nc.scalar.copy(wbc[:, :W], pwb[:, :W])
                if b >= 1:
                    emit_attend(b - 1)
            emit_attend(BL - 1)

            # ======== finale ==============================================
            attended = cw.tile([BL, HID], BF16)
            for mt in range(4):
                pt8 = psmall.tile([BL, 128], F32, tag="small")
                nc.tensor.transpose(pt8[:], attT[:, mt, :], identf[:, :])
                nc.vector.tensor_copy(attended[:, mt * 128:(mt + 1) * 128],
                                      pt8[:])
            pa48 = pacc.tile([BK, HID], F32, tag="acc")
            nc.tensor.matmul(pa48[:], a48[:], attended[:],
                             start=True, stop=True)
            scr48b = cw.tile([BK, HID], F32)
            dotraw = cw.tile([BK, 1], F32)
            nc.vector.tensor_tensor(scr48b[:], sep[:], pa48[:], op=OP.mult)
            nc.vector.tensor_reduce(dotraw[:], scr48b[:],
                                    axis=mybir.AxisListType.X, op=OP.add)
            dotf = cw.tile([BK, 1], F32)
            nc.vector.tensor_scalar_mul(dotf[:], dotraw[:], rnorm[:])
            nc.sync.dma_start(d_out.ap(), dotf[:])

    nc.compile()
    return nc


def _get_nc(widths):
    key = tuple(widths)
    if key not in _NC_CACHE:
        _NC_CACHE[key] = _build_nc(key)
    return _NC_CACHE[key]


def _t128(w, a):
    h = w.shape[1]
    return np.ascontiguousarray(
        w.astype(BF).reshape(a, 128, h).transpose(1, 0, 2)).reshape(128, a * h)


def _plan(masks):
    nk = (~masks.reshape(B, S)).sum(1)
    perm = np.argsort(-nk, kind="stable")
    widths = []
    for s in range(BL):
        w = int(nk[perm[s * NCORES]])
        w = min(max((w + 15) // 16 * 16, 16), S)
        widths.append(w)
    return perm, tuple(widths), nk


def _make_in_maps(inputs, perm, widths):
    reps = np.asarray(inputs["representations"], dtype=np.float32)
    si = np.asarray(inputs["separate_images"], dtype=np.float32)
    vc = np.asarray(inputs["visual_context"], dtype=np.float32)
    ph = np.asarray(inputs["prev_hist"], dtype=np.float32)
    cnts = np.asarray(inputs["hist_counts"]).astype(np.float32)
    msks = np.asarray(inputs["masks"]).astype(bool).reshape(B, S)
    SW = sum(widths)
    cum = np.concatenate([[0], np.cumsum(widths)]).astype(int)

    bias = np.concatenate([
        np.asarray(inputs["b_e2h"], np.float32).reshape(4, 128),
        np.asarray(inputs["b_mm"], np.float32).reshape(4, 128),
        np.asarray(inputs["b_a1"], np.float32).reshape(4, 128)], 0).T
    rows = np.concatenate([
        np.asarray(inputs["b_ctx"], np.float32),
        np.asarray(inputs["b_sep"], np.float32),
        np.asarray(inputs["b_hist"], np.float32)]).reshape(1, 3 * HID)
    a48 = (np.arange(BK)[None, :] // K6 ==
           np.arange(BL)[:, None]).astype(np.float32)

    shared = {
        "we2h": _t128(np.asarray(inputs["W_e2h"], np.float32), K6),
        "wmm": _t128(np.asarray(inputs["W_mm"], np.float32), 8),
        "wa1": _t128(np.asarray(inputs["W_a1"], np.float32), 4),
        "wa2": np.ascontiguousarray(
            np.asarray(inputs["W_a2"], np.float32).reshape(4, 128).T
        ).astype(BF),
        "whist": _t128(np.asarray(inputs["W_hist"], np.float32), K6),
        "wsep": _t128(np.asarray(inputs["W_sep"], np.float32), 16),
        "bias": np.ascontiguousarray(bias),
        "rows": rows.astype(BF),
        "a48": a48.astype(BF),
    }
    shared["wctx"] = _t128(np.asarray(inputs["W_ctx"], np.float32), 96)

    in_maps = []
    for c in range(NCORES):
        gb = [int(perm[s * NCORES + c]) for s in range(BL)]  # slot -> batch
        m = dict(shared)
        xtc = np.zeros((128, K6 * SW), dtype=BF)
        mt = np.zeros((1, SW), dtype=np.float32)
        for s, g in enumerate(gb):
            W = widths[s]
            keep = np.flatnonzero(~msks[g])
            k = min(len(keep), W)
            arr = np.zeros((W, EMB), dtype=np.float32)
            arr[:k] = reps[g, keep[:k]]
            blk = arr.astype(BF).reshape(W, K6, 128).transpose(2, 1, 0)
            xtc[:, K6 * cum[s]:K6 * cum[s + 1]] = blk.reshape(128, K6 * W)
            mt[0, cum[s]:cum[s] + W] = np.where(np.arange(W) < k, 0.0, -1e30)
        m["xt"] = np.ascontiguousarray(xtc)
        m["mterm"] = mt.astype(BF)
        m["sit"] = _t128(si[gb].reshape(BK, IMG).T.copy(), 16)
        m["vct"] = np.ascontiguousarray(
            vc[gb].T.astype(BF).reshape(96, 128, BL)
            .transpose(1, 0, 2)).reshape(128, 96 * BL)
        m["ph"] = np.ascontiguousarray(
            ph[gb].astype(BF).reshape(BK, HL, 2, EMB // 2)
            .transpose(2, 0, 1, 3)).reshape(2 * BK, HL * (EMB // 2))
        cnt = cnts[gb].reshape(BK)
        valid = (np.arange(HL)[None, :] < cnt[:, None]).astype(np.float32)
        w48 = valid / np.maximum(cnt, 1.0)[:, None]
        m["w96"] = np.ascontiguousarray(np.tile(w48, (2, 1)))
        m["g48"] = (cnt > 0).astype(np.float32).reshape(1, BK).astype(BF)
        in_maps.append(m)
    return in_maps


def run(inputs, trace=False, trace_kwargs={}, run_kwargs={}):
    masks = np.asarray(inputs["masks"]).astype(bool)
    perm, widths, nk = _plan(masks)
    nc = _get_nc(widths)
    in_maps = _make_in_maps(inputs, perm, widths)
    res = run_bass_kernel_spmd(nc, in_maps, core_ids=list(range(NCORES)),
                               trace=trace, trace_kwargs=trace_kwargs,
                               **run_kwargs)
    out = np.zeros((B, K6, 1), dtype=np.float32)
    for c in range(NCORES):
        oc = res.results[c]["out"].reshape(BL, K6)
        for s in range(BL):
            out[perm[s * NCORES + c], :, 0] = oc[s]
    return out, res


def kernel(**inputs):
    out, _ = run(inputs, trace=False)
    return out
